# revision 19
# baseline (speedup 1.0000x reference)
"""Trainium2 Bass kernel for nn_SSLModel (dual-branch 3-layer GCN + segment-max pool + MLP head).

Strategy (8 NeuronCores):
  - Cores 0-3 run branch s, cores 4-7 run branch t (one SPMD program, per-core data).
  - Node rows are kept in input order (batch vector is sorted, so rows are already
    grouped by graph). Two row spaces:
      U: raw node rows padded to 10240 (128*80); each core owns 20 blocks of 128.
         conv1/conv2 destination blocks live here (no per-graph padding at all).
      P: graph-padded-to-16 rows (for pooling); each core owns P_BLOCKS blocks.
         conv3 destination blocks live here so pooling can use uniform 16-row
         segment reduces.
  - gcn_conv = Ahat @ (x) @ W computed aggregation-first:
      y = Ahat @ x via per-128-edge-chunk indirect row gathers (one offset per
      partition - hardware supports exactly this) + one-hot "T matrix" matmuls
      on the tensor engine (scatter-add as matmul). Chunk counts per block are
      variable; blocks are processed in per-core sorted order against a shared
      max-schedule so the SPMD program is identical across cores.
  - All aggregation/transform matmuls run in bf16 (ends up ~4e-3 rel err, gate
    is 2e-2); activations stored in DRAM as bf16 halving gather traffic.
  - conv3 produces z^T (features on partitions), tensor_reduce(max) over 16-row
    segments -> per-core segment maxes; one 8-core AllGather of segment maxes;
    per-graph fold via data-driven indirect gathers + DVE max; MLP head
    computed redundantly on every core.
"""
import sys
import numpy as np
import ml_dtypes

sys.path.insert(0, "/opt/trn_rl_repo")

from contextlib import ExitStack

import concourse.bass as bass
import concourse.tile as tile
from concourse import bacc, mybir
from concourse.masks import make_identity

N_NODES = 10000
E_EDGES = 160000
G_GRAPHS = 128
D_IN, D1, D2, D3, DH, D_OUT = 128, 512, 1024, 2048, 1024, 1317
NCORES = 8
CPBRANCH = 4
GROUPS4 = [[0, 1, 2, 3], [4, 5, 6, 7]]
GROUPS8 = [[0, 1, 2, 3, 4, 5, 6, 7]]
U_TOTAL = 10240          # 128 * 80
U_BLOCKS = U_TOTAL // 128 // CPBRANCH   # 20 per core
SEG = 16                 # pooling segment granularity

f32 = mybir.dt.float32
f32r = mybir.dt.float32r
bf16 = mybir.dt.bfloat16
i32 = mybir.dt.int32
RELU = mybir.ActivationFunctionType.Relu
SIGM = mybir.ActivationFunctionType.Sigmoid
MAX = mybir.AluOpType.max


# ----------------------------------------------------------------------------- host prep
def _prep_branch(x, edge_index, batch, p_blocks_core=None):
    """Per-branch host packing. Returns dict with per-core T/idx arrays etc."""
    batch = np.asarray(batch, np.int64)
    n = batch.shape[0]
    order = np.argsort(batch, kind="stable")          # identity when sorted
    inv = np.empty(n, np.int64)
    inv[order] = np.arange(n)
    xs = np.asarray(x, np.float32)[order]

    counts = np.bincount(batch, minlength=G_GRAPHS)
    # P space: graphs padded to SEG
    pcnt = -(-counts // SEG) * SEG
    pstart = np.concatenate([[0], np.cumsum(pcnt)[:-1]])
    p_used = int(pcnt.sum())

    src = inv[np.asarray(edge_index[0], np.int64)]
    dst = inv[np.asarray(edge_index[1], np.int64)]
    src = np.concatenate([src, np.arange(n)])
    dst = np.concatenate([dst, np.arange(n)])
    deg = np.bincount(dst, minlength=n).astype(np.float64)
    dinv = 1.0 / np.sqrt(deg)
    norm = (dinv[src] * dinv[dst]).astype(np.float32)

    ustart = np.concatenate([[0], np.cumsum(counts)[:-1]])
    pdst = pstart[batch[dst]] + (dst - ustart[batch[dst]])

    return dict(xs=xs, src=src, dst=dst, pdst=pdst, norm=norm,
                counts=counts, pcnt=pcnt, pstart=pstart, p_used=p_used)


def _bucket(dstrows, ncore_rows):
    """Sort edges by dst block; per-core sorted block order + chunk counts."""
    blk = dstrows // 128
    eorder = np.argsort(blk, kind="stable")
    nblocks = CPBRANCH * (ncore_rows // 128)
    bc = np.bincount(blk[eorder], minlength=nblocks)
    return eorder, bc


def _pack_core(eorder, bc, src, dstrows, norm, core, blocks_core, sched):
    """Pack one core's T/idx in sorted-block position order against sched."""
    bstart = np.concatenate([[0], np.cumsum(bc)])
    lo_b = core * blocks_core
    myblocks = np.arange(lo_b, lo_b + blocks_core)
    order = np.argsort(-bc[myblocks], kind="stable")  # positions -> block
    nch = int(np.sum(sched))
    T = np.zeros((128, nch * 128), np.float32)
    idx = np.zeros((128, nch), np.int32)
    ck = 0
    blkperm = np.empty(blocks_core, np.int64)
    for pos in range(blocks_core):
        b = myblocks[order[pos]]
        blkperm[pos] = b
        es = eorder[bstart[b]:bstart[b + 1]]
        ne = len(es)
        j = np.arange(ne)
        c = j // 128
        s = j % 128
        T[s, (ck + c) * 128 + (dstrows[es] % 128)] = norm[es]
        idx[s, ck + c] = src[es]
        ck += int(sched[pos])
    return T.astype(ml_dtypes.bfloat16), idx, blkperm


def _schedules(bcs, blocks_core):
    """Shared per-position chunk schedule = max over the 8 core-shards."""
    prof = []
    for bc in bcs:  # one bc per (branch, core)
        for core in range(CPBRANCH):
            mine = bc[core * blocks_core:(core + 1) * blocks_core]
            prof.append(np.sort(-(-mine // 128))[::-1])
    sched = np.maximum.reduce(prof)
    return np.maximum(sched, 1).astype(np.int64)


# ----------------------------------------------------------------------------- program
def build_nc(sched12, sched3, p_blocks, J, repeat=1, stages=6):
    sched12, sched3 = list(sched12), list(sched3)
    NCH12, NCH3 = sum(sched12), sum(sched3)
    P_CORE = p_blocks * 128            # P rows per core
    NSEG = P_CORE // SEG               # segs per core
    SEGALL = NCORES * NSEG             # rows in allgathered segmax
    UB = U_BLOCKS
    U_CORE = UB * 128

    nc = bacc.Bacc("TRN2", target_bir_lowering=False, debug=False, num_devices=NCORES)

    x_in = nc.dram_tensor("x", [U_TOTAL, D_IN], bf16, kind="ExternalInput")
    T12_in = nc.dram_tensor("T12", [128, NCH12 * 128], bf16, kind="ExternalInput")
    idx12_in = nc.dram_tensor("idx12", [128, NCH12], i32, kind="ExternalInput")
    idx12p_in = nc.dram_tensor("idx12p", [128, NCH12], i32, kind="ExternalInput")
    T3_in = nc.dram_tensor("T3", [128, NCH3 * 128], bf16, kind="ExternalInput")
    idx3_in = nc.dram_tensor("idx3", [128, NCH3], i32, kind="ExternalInput")
    fl3_in = nc.dram_tensor("fl3", [1, P_CORE], bf16, kind="ExternalInput")
    fidx_in = nc.dram_tensor("fidx", [128, 2 * J], i32, kind="ExternalInput")
    W1_in = nc.dram_tensor("W1", [D_IN, D1], bf16, kind="ExternalInput")
    b1_in = nc.dram_tensor("b1", [1, D1], bf16, kind="ExternalInput")
    W2_in = nc.dram_tensor("W2", [D1, D2], bf16, kind="ExternalInput")
    b2_in = nc.dram_tensor("b2", [1, D2], bf16, kind="ExternalInput")
    W3_in = nc.dram_tensor("W3", [D2, D3], bf16, kind="ExternalInput")
    b3_in = nc.dram_tensor("b3", [1, D3], bf16, kind="ExternalInput")
    Wl1_in = nc.dram_tensor("Wl1", [D3, DH], bf16, kind="ExternalInput")
    bl1_in = nc.dram_tensor("bl1", [1, DH], f32, kind="ExternalInput")
    Wl2_in = nc.dram_tensor("Wl2", [DH, D_OUT], bf16, kind="ExternalInput")
    bl2_in = nc.dram_tensor("bl2", [1, D_OUT], f32, kind="ExternalInput")

    out_z = nc.dram_tensor("out_z", [G_GRAPHS, D_OUT], f32, kind="ExternalOutput")
    out_sig = nc.dram_tensor("out_sig", [G_GRAPHS, D_OUT], f32, kind="ExternalOutput")

    act1_loc = nc.dram_tensor("act1_loc", [U_CORE, D1], bf16)
    act1 = nc.dram_tensor("act1", [CPBRANCH * U_CORE, D1], bf16)
    act2_loc = nc.dram_tensor("act2_loc", [U_CORE, D2], bf16)
    act2 = nc.dram_tensor("act2", [CPBRANCH * U_CORE, D2], bf16)
    seg_loc = nc.dram_tensor("seg_loc", [NSEG, D3], bf16)
    seg_all = nc.dram_tensor("seg_all", [SEGALL + 1, D3], bf16, addr_space="Shared")

    dram = dict(x=x_in, T12=T12_in, idx12=idx12_in, idx12p=idx12p_in,
                T3=T3_in, idx3=idx3_in,
                fl3=fl3_in, fidx=fidx_in,
                W1=W1_in, b1=b1_in, W2=W2_in, b2=b2_in, W3=W3_in, b3=b3_in,
                Wl1=Wl1_in, bl1=bl1_in, Wl2=Wl2_in, bl2=bl2_in,
                out_z=out_z, out_sig=out_sig,
                act1_loc=act1_loc, act1=act1, act2_loc=act2_loc, act2=act2,
                seg_loc=seg_loc, seg_all=seg_all)

    with tile.TileContext(nc) as tc:
        with ExitStack() as rctx:
            res = rctx.enter_context(tc.tile_pool(name="res", bufs=1))
            ident = res.tile([128, 128], f32)
            make_identity(nc, ident[:])
            ident_bf = res.tile([128, 128], bf16)
            nc.scalar.copy(out=ident_bf[:], in_=ident[:])
            ones_bf = res.tile([1, 128], bf16)
            nc.vector.memset(ones_bf[:], 1.0)
            for _rep in range(repeat):
                _emit(nc, tc, dram, sched12, sched3, p_blocks, J,
                      ident, ident_bf, ones_bf, stages)

    nc.compile()
    return nc


def _dummy_outputs(nc, tc, dram):
    with ExitStack() as ctx:
        sb = ctx.enter_context(tc.tile_pool(name="dout", bufs=1))
        t = sb.tile([128, D_OUT], f32)
        nc.vector.memset(t[:], 0.0)
        nc.sync.dma_start(out=dram["out_z"][:, :], in_=t[:])
        nc.sync.dma_start(out=dram["out_sig"][:, :], in_=t[:])


def _emit(nc, tc, dram, sched12, sched3, p_blocks, J, ident, ident_bf, ones_bf,
          stages):
    AG = "AllGather"
    BYP = mybir.AluOpType.bypass
    UB = U_BLOCKS
    NSEG = p_blocks * 128 // SEG
    SEGALL = NCORES * NSEG

    # ---------------- conv1: x[U,128] -> act1_loc[U_CORE, 512]
    if stages >= 1:
        with ExitStack() as ctx:
            wb = ctx.enter_context(tc.tile_pool(name="c1wb", bufs=1))
            sb = ctx.enter_context(tc.tile_pool(name="c1sb", bufs=3))
            ps = ctx.enter_context(tc.tile_pool(name="c1ps", bufs=2, space="PSUM"))
            W1_sb = wb.tile([128, D1], bf16)
            nc.sync.dma_start(out=W1_sb[:], in_=dram["W1"][:, :])
            b1_sb = wb.tile([1, D1], bf16)
            nc.sync.dma_start(out=b1_sb[:], in_=dram["b1"][:, :])
            ck = 0
            for k in range(UB):
                sk = sched12[k]
                T_sb = sb.tile([128, sk * 128], bf16, tag="T")
                nc.sync.dma_start(out=T_sb[:], in_=dram["T12"][:, ck * 128:(ck + sk) * 128])
                ix = sb.tile([128, sk], i32, tag="ix")
                nc.sync.dma_start(out=ix[:], in_=dram["idx12"][:, ck:ck + sk])
                G = sb.tile([128, sk * D_IN], bf16, tag="G")
                for j in range(sk):
                    nc.gpsimd.indirect_dma_start(
                        out=G[:, j * D_IN:(j + 1) * D_IN], out_offset=None,
                        in_=dram["x"][:, :],
                        in_offset=bass.IndirectOffsetOnAxis(ap=ix[:, j:j + 1], axis=0))
                y_ps = ps.tile([128, D_IN], f32, tag="y")
                for j in range(sk):
                    nc.tensor.matmul(out=y_ps[:], lhsT=T_sb[:, j * 128:(j + 1) * 128],
                                     rhs=G[:, j * D_IN:(j + 1) * D_IN],
                                     start=(j == 0), stop=(j == sk - 1))
                y_sb = sb.tile([128, D_IN], bf16, tag="ys")
                nc.scalar.copy(out=y_sb[:], in_=y_ps[:])
                t_ps = ps.tile([128, 128], bf16, tag="tp")
                nc.tensor.transpose(out=t_ps[:], in_=y_sb[:], identity=ident_bf[:])
                yt = sb.tile([128, D_IN], bf16, tag="yt")
                nc.scalar.copy(out=yt[:], in_=t_ps[:])
                z_ps = ps.tile([128, D1], f32, tag="z")
                nc.tensor.matmul(out=z_ps[:], lhsT=yt[:], rhs=W1_sb[:],
                                 start=True, stop=False)
                nc.tensor.matmul(out=z_ps[:], lhsT=ones_bf[:], rhs=b1_sb[:],
                                 start=False, stop=True)
                z_sb = sb.tile([128, D1], bf16, tag="zs")
                nc.scalar.copy(out=z_sb[:], in_=z_ps[:])
                nc.sync.dma_start(out=dram["act1_loc"][k * 128:(k + 1) * 128, :], in_=z_sb[:])
                ck += sk
    if stages < 2:
        _dummy_outputs(nc, tc, dram)
        return
    nc.gpsimd.collective_compute(AG, BYP, ins=[dram["act1_loc"][:, :]],
                                 outs=[dram["act1"][:, :]], replica_groups=GROUPS4)

    # ---------------- conv2: act1 -> act2_loc[U_CORE, 1024] (relu)
    if stages >= 3:
        g2span = 5
        with ExitStack() as ctx:
            wb = ctx.enter_context(tc.tile_pool(name="c2wb", bufs=1))
            sb = ctx.enter_context(tc.tile_pool(name="c2sb", bufs=2))
            ps = ctx.enter_context(tc.tile_pool(name="c2ps", bufs=2, space="PSUM"))
            W2_sb = [wb.tile([128, D2], bf16, tag=f"W2_{k}", name=f"W2_{k}")
                     for k in range(D1 // 128)]
            for k in range(D1 // 128):
                nc.sync.dma_start(out=W2_sb[k][:], in_=dram["W2"][k * 128:(k + 1) * 128, :])
            b2_sb = wb.tile([1, D2], bf16)
            nc.sync.dma_start(out=b2_sb[:], in_=dram["b2"][:, :])
            ck = 0
            for k in range(UB):
                sk = sched12[k]
                T_sb = sb.tile([128, sk * 128], bf16, tag="T")
                nc.sync.dma_start(out=T_sb[:], in_=dram["T12"][:, ck * 128:(ck + sk) * 128])
                ix = sb.tile([128, sk], i32, tag="ix")
                nc.sync.dma_start(out=ix[:], in_=dram["idx12p"][:, ck:ck + sk])
                n_g = -(-sk // g2span)
                Gs = []
                for gi in range(n_g):
                    lo = gi * g2span
                    hi = min(sk, lo + g2span)
                    G = sb.tile([128, g2span * D1], bf16, tag=f"G{gi}", name=f"G{gi}")
                    for c in range(lo, hi):
                        nc.gpsimd.indirect_dma_start(
                            out=G[:, (c - lo) * D1:(c - lo + 1) * D1], out_offset=None,
                            in_=dram["act1"][:, :],
                            in_offset=bass.IndirectOffsetOnAxis(ap=ix[:, c:c + 1], axis=0))
                    Gs.append(G)
                y_ps = ps.tile([128, D1], f32, tag="y")
                for c in range(sk):
                    G = Gs[c // g2span]
                    co = c % g2span
                    nc.tensor.matmul(out=y_ps[:], lhsT=T_sb[:, c * 128:(c + 1) * 128],
                                     rhs=G[:, co * D1:(co + 1) * D1],
                                     start=(c == 0), stop=(c == sk - 1))
                y_sb = sb.tile([128, D1], bf16, tag="ys")
                nc.scalar.copy(out=y_sb[:], in_=y_ps[:])
                yt = sb.tile([128, D1], bf16, tag="yt")
                for q in range(D1 // 128):
                    t_ps = ps.tile([128, 128], bf16, tag="tp")
                    nc.tensor.transpose(out=t_ps[:], in_=y_sb[:, q * 128:(q + 1) * 128],
                                        identity=ident_bf[:])
                    nc.scalar.copy(out=yt[:, q * 128:(q + 1) * 128], in_=t_ps[:])
                z_ps = ps.tile([128, D2], f32, tag="z")
                for nn in range(D2 // 512):
                    nsl = slice(nn * 512, (nn + 1) * 512)
                    for q in range(D1 // 128):
                        nc.tensor.matmul(out=z_ps[:, nsl],
                                         lhsT=yt[:, q * 128:(q + 1) * 128],
                                         rhs=W2_sb[q][:, nsl],
                                         start=(q == 0), stop=False)
                    nc.tensor.matmul(out=z_ps[:, nsl], lhsT=ones_bf[:],
                                     rhs=b2_sb[:, nsl], start=False, stop=True)
                z_sb = sb.tile([128, D2], bf16, tag="zs")
                nc.scalar.activation(out=z_sb[:], in_=z_ps[:], func=RELU)
                nc.sync.dma_start(out=dram["act2_loc"][k * 128:(k + 1) * 128, :], in_=z_sb[:])
                ck += sk
    if stages < 4:
        _dummy_outputs(nc, tc, dram)
        return
    nc.gpsimd.collective_compute(AG, BYP, ins=[dram["act2_loc"][:, :]],
                                 outs=[dram["act2"][:, :]], replica_groups=GROUPS4)

    # ---------------- conv3 (transposed output) + 16-row segment max
    if stages >= 5:
        g3span = 3
        GRP = 4  # P-blocks per W3 batch
        with ExitStack() as ctx:
            wb = ctx.enter_context(tc.tile_pool(name="c3wb", bufs=1))
            sb = ctx.enter_context(tc.tile_pool(name="c3sb", bufs=2))
            ps = ctx.enter_context(tc.tile_pool(name="c3ps", bufs=2, space="PSUM"))
            tps = ctx.enter_context(tc.tile_pool(name="c3tps", bufs=2, space="PSUM"))
            zps = ctx.enter_context(tc.tile_pool(name="c3zps", bufs=2, space="PSUM"))
            pool_res = ctx.enter_context(tc.tile_pool(name="poolres", bufs=1))
            W3_sb = [wb.tile([128, D3], bf16, tag=f"W3_{k}", name=f"W3_{k}")
                     for k in range(D2 // 128)]
            for k in range(D2 // 128):
                nc.sync.dma_start(out=W3_sb[k][:], in_=dram["W3"][k * 128:(k + 1) * 128, :])
            b3_sb = wb.tile([1, D3], bf16)
            nc.sync.dma_start(out=b3_sb[:], in_=dram["b3"][:, :])
            poolseg = [pool_res.tile([128, NSEG], f32, tag=f"pseg{oc}", name=f"pseg{oc}")
                       for oc in range(D3 // 128)]
            ck = 0
            q_lo = 0
            while q_lo < p_blocks:
                q_n = min(GRP, p_blocks - q_lo)
                cols = q_n * 128
                yt_grp = [sb.tile([128, GRP * 128], bf16, tag=f"ytg{k}", name=f"ytg{k}")
                          for k in range(D2 // 128)]
                for jb in range(q_n):
                    kblk = q_lo + jb
                    sk = sched3[kblk]
                    T_sb = sb.tile([128, sk * 128], bf16, tag="T")
                    nc.sync.dma_start(out=T_sb[:], in_=dram["T3"][:, ck * 128:(ck + sk) * 128])
                    ix = sb.tile([128, sk], i32, tag="ix")
                    nc.sync.dma_start(out=ix[:], in_=dram["idx3"][:, ck:ck + sk])
                    n_g = -(-sk // g3span)
                    Gs = []
                    for gi in range(n_g):
                        lo = gi * g3span
                        hi = min(sk, lo + g3span)
                        G = sb.tile([128, g3span * D2], bf16, tag=f"G{gi}", name=f"G{gi}")
                        for c in range(lo, hi):
                            nc.gpsimd.indirect_dma_start(
                                out=G[:, (c - lo) * D2:(c - lo + 1) * D2], out_offset=None,
                                in_=dram["act2"][:, :],
                                in_offset=bass.IndirectOffsetOnAxis(ap=ix[:, c:c + 1], axis=0))
                        Gs.append(G)
                    y_ps = ps.tile([128, D2], f32, tag="y")
                    for c in range(sk):
                        G = Gs[c // g3span]
                        co = c % g3span
                        for nn in range(D2 // 512):
                            nc.tensor.matmul(
                                out=y_ps[:, nn * 512:(nn + 1) * 512],
                                lhsT=T_sb[:, c * 128:(c + 1) * 128],
                                rhs=G[:, co * D2 + nn * 512:co * D2 + (nn + 1) * 512],
                                start=(c == 0), stop=(c == sk - 1))
                    y_sb = sb.tile([128, D2], bf16, tag="ys")
                    nc.scalar.copy(out=y_sb[:], in_=y_ps[:])
                    for k in range(D2 // 128):
                        t_ps = tps.tile([128, 128], bf16, tag="tp")
                        nc.tensor.transpose(out=t_ps[:], in_=y_sb[:, k * 128:(k + 1) * 128],
                                            identity=ident_bf[:])
                        nc.scalar.copy(out=yt_grp[k][:, jb * 128:(jb + 1) * 128], in_=t_ps[:])
                    ck += sk
                fl = sb.tile([1, GRP * 128], bf16, tag="fl")
                nc.sync.dma_start(out=fl[:, :cols],
                                  in_=dram["fl3"][0:1, q_lo * 128:q_lo * 128 + cols])
                for oc in range(D3 // 128):
                    zt_ps = zps.tile([128, GRP * 128], f32, tag="zt")
                    for k in range(D2 // 128):
                        nc.tensor.matmul(out=zt_ps[:, :cols],
                                         lhsT=W3_sb[k][:, oc * 128:(oc + 1) * 128],
                                         rhs=yt_grp[k][:, :cols],
                                         start=(k == 0), stop=False)
                    nc.tensor.matmul(out=zt_ps[:, :cols],
                                     lhsT=b3_sb[:, oc * 128:(oc + 1) * 128],
                                     rhs=fl[:, :cols], start=False, stop=True)
                    nc.vector.tensor_reduce(
                        out=poolseg[oc][:, q_lo * 8:q_lo * 8 + cols // SEG],
                        in_=zt_ps[:, :cols].rearrange("p (s n) -> p s n", n=SEG),
                        axis=mybir.AxisListType.X, op=MAX)
                q_lo += q_n

            # transpose poolseg [128, NSEG] x16 -> seg rows (bf16), write seg_loc
            pb = [pool_res.tile([128, NSEG], bf16, tag=f"pb{oc}", name=f"pb{oc}")
                  for oc in range(D3 // 128)]
            for oc in range(D3 // 128):
                nc.scalar.copy(out=pb[oc][:], in_=poolseg[oc][:])
            for lo in range(0, NSEG, 128):
                w = min(128, NSEG - lo)
                out_sb = pool_res.tile([128, D3], bf16, tag="segT", name=f"segT{lo}")
                for oc in range(D3 // 128):
                    t_ps = tps.tile([128, 128], bf16, tag="tp")
                    nc.tensor.transpose(out=t_ps[:w, :], in_=pb[oc][:, lo:lo + w],
                                        identity=ident_bf[:])
                    nc.scalar.copy(out=out_sb[:w, oc * 128:(oc + 1) * 128], in_=t_ps[:w, :])
                nc.sync.dma_start(out=dram["seg_loc"][lo:lo + w, :], in_=out_sb[:w, :])
    if stages < 6:
        _dummy_outputs(nc, tc, dram)
        return

    # zero the dummy fold row, then allgather segment maxes
    with ExitStack() as ctx:
        zb = ctx.enter_context(tc.tile_pool(name="zrow", bufs=1))
        zt = zb.tile([1, D3], bf16)
        nc.vector.memset(zt[:], 0.0)
        nc.sync.dma_start(out=dram["seg_all"][SEGALL:SEGALL + 1, :], in_=zt[:])
    nc.gpsimd.collective_compute(AG, BYP, ins=[dram["seg_loc"][:, :]],
                                 outs=[dram["seg_all"][0:SEGALL, :]],
                                 replica_groups=GROUPS8)

    # ---------------- per-graph fold + head (every core)
    with ExitStack() as ctx:
        sb = ctx.enter_context(tc.tile_pool(name="hsb", bufs=2))
        wsb = ctx.enter_context(tc.tile_pool(name="hwsb", bufs=2))
        ps = ctx.enter_context(tc.tile_pool(name="hps", bufs=1, space="PSUM"))
        fidx = sb.tile([128, 2 * J], i32, tag="fidx")
        nc.sync.dma_start(out=fidx[:], in_=dram["fidx"][:, :])
        pool_s = sb.tile([128, D3], bf16, tag="ps")
        pool_t = sb.tile([128, D3], bf16, tag="pt")
        nc.vector.memset(pool_s[:], 0.0)
        nc.vector.memset(pool_t[:], 0.0)
        for j in range(2 * J):
            gt = sb.tile([128, D3], bf16, tag="gt")
            nc.gpsimd.indirect_dma_start(
                out=gt[:], out_offset=None, in_=dram["seg_all"][:, :],
                in_offset=bass.IndirectOffsetOnAxis(ap=fidx[:, j:j + 1], axis=0))
            acc = pool_s if j < J else pool_t
            nc.vector.tensor_max(out=acc[:], in0=acc[:], in1=gt[:])
        z_sb = sb.tile([128, D3], bf16, tag="zsum")
        nc.vector.tensor_add(out=z_sb[:], in0=pool_s[:], in1=pool_t[:])

        bl1f = wsb.tile([1, DH], f32)
        nc.sync.dma_start(out=bl1f[:], in_=dram["bl1"][:, :])
        bl1_sb = wsb.tile([1, DH], bf16)
        nc.scalar.copy(out=bl1_sb[:], in_=bl1f[:])
        bl2f = wsb.tile([1, D_OUT], f32)
        nc.sync.dma_start(out=bl2f[:], in_=dram["bl2"][:, :])
        bl2_sb = wsb.tile([1, D_OUT], bf16)
        nc.scalar.copy(out=bl2_sb[:], in_=bl2f[:])
        ones_f = wsb.tile([1, 128], bf16)
        nc.vector.memset(ones_f[:], 1.0)

        zT = sb.tile([128, D3], bf16, tag="zT")
        for k in range(D3 // 128):
            t_ps = ps.tile([128, 128], bf16, tag="tp")
            nc.tensor.transpose(out=t_ps[:], in_=z_sb[:, k * 128:(k + 1) * 128],
                                identity=ident_bf[:])
            nc.scalar.copy(out=zT[:, k * 128:(k + 1) * 128], in_=t_ps[:])
        h_ps = ps.tile([128, DH], f32, tag="h")
        for k in range(D3 // 128):
            wl1 = wsb.tile([128, DH], bf16, tag="wl1")
            nc.sync.dma_start(out=wl1[:], in_=dram["Wl1"][k * 128:(k + 1) * 128, :])
            for nn in range(DH // 512):
                nsl = slice(nn * 512, (nn + 1) * 512)
                nc.tensor.matmul(out=h_ps[:, nsl], lhsT=zT[:, k * 128:(k + 1) * 128],
                                 rhs=wl1[:, nsl], start=(k == 0), stop=False)
        for nn in range(DH // 512):
            nsl = slice(nn * 512, (nn + 1) * 512)
            nc.tensor.matmul(out=h_ps[:, nsl], lhsT=ones_f[:],
                             rhs=bl1_sb[:, nsl], start=False, stop=True)
        h_sb = sb.tile([128, DH], bf16, tag="hs")
        nc.scalar.activation(out=h_sb[:], in_=h_ps[:], func=RELU)
        hT = sb.tile([128, DH], bf16, tag="hT")
        for k in range(DH // 128):
            t_ps = ps.tile([128, 128], bf16, tag="tp")
            nc.tensor.transpose(out=t_ps[:], in_=h_sb[:, k * 128:(k + 1) * 128],
                                identity=ident_bf[:])
            nc.scalar.copy(out=hT[:, k * 128:(k + 1) * 128], in_=t_ps[:])
        z_out = sb.tile([128, D_OUT], f32, tag="zo")
        sig = sb.tile([128, D_OUT], f32, tag="sg")
        o_ps = ps.tile([128, D_OUT], f32, tag="o")
        for k in range(DH // 128):
            wl2 = wsb.tile([128, D_OUT], bf16, tag="wl2")
            nc.sync.dma_start(out=wl2[:], in_=dram["Wl2"][k * 128:(k + 1) * 128, :])
            for (lo, hi) in [(0, 512), (512, 1024), (1024, D_OUT)]:
                nc.tensor.matmul(out=o_ps[:, lo:hi], lhsT=hT[:, k * 128:(k + 1) * 128],
                                 rhs=wl2[:, lo:hi], start=(k == 0), stop=False)
        for (lo, hi) in [(0, 512), (512, 1024), (1024, D_OUT)]:
            nc.tensor.matmul(out=o_ps[:, lo:hi], lhsT=ones_f[:],
                             rhs=bl2_sb[:, lo:hi], start=False, stop=True)
        nc.scalar.copy(out=z_out[:], in_=o_ps[:])
        nc.scalar.activation(out=sig[:], in_=z_out[:], func=SIGM)
        nc.sync.dma_start(out=dram["out_z"][:, :], in_=z_out[:])
        nc.sync.dma_start(out=dram["out_sig"][:, :], in_=sig[:])


# ----------------------------------------------------------------------------- driver
_PROGRAM_CACHE = {}
_IDENT_BF = [None]


def _get_program(key, repeat=1, stages=6):
    import os
    stages = int(os.environ.get("KSTAGES", stages))
    sched12, sched3, p_blocks, J = key
    k = (key, repeat, stages)
    if k not in _PROGRAM_CACHE:
        _PROGRAM_CACHE[k] = build_nc(sched12, sched3, p_blocks, J,
                                     repeat=repeat, stages=stages)
    return _PROGRAM_CACHE[k]


def make_in_maps(x_s, x_t, W_enc1, b_enc1, W_enc2, b_enc2,
                 W_r1g1, b_r1g1, W_r1g2, b_r1g2,
                 W_r2g1, b_r2g1, W_r2g2, b_r2g2,
                 W_l1, b_l1, W_l2, b_l2,
                 edge_index_s, edge_index_t, xs_batch, xt_batch):
    brs = _prep_branch(x_s, edge_index_s, xs_batch)
    brt = _prep_branch(x_t, edge_index_t, xt_batch)

    # common P geometry
    p_core = 128 * int(np.ceil(max(brs["p_used"], brt["p_used"]) / (128 * CPBRANCH)))
    p_blocks = p_core // 128
    NSEG = p_core // SEG

    # buckets + schedules
    b12, b3 = [], []
    for br in (brs, brt):
        eo12, bc12 = _bucket(br["dst"], U_TOTAL // CPBRANCH)
        eo3, bc3 = _bucket(br["pdst"], p_core)
        br["eo12"], br["bc12"], br["eo3"], br["bc3"] = eo12, bc12, eo3, bc3
        b12.append(bc12)
        b3.append(bc3)
    sched12 = _schedules(b12, U_BLOCKS)
    sched3 = _schedules(b3, p_blocks)

    J = max(1, int(max((-(-brs["counts"] // SEG)).max(),
                       (-(-brt["counts"] // SEG)).max())))
    SEGALL = NCORES * NSEG

    bf = lambda a: np.ascontiguousarray(np.asarray(a, np.float32).astype(ml_dtypes.bfloat16))
    f32c = lambda a: np.ascontiguousarray(np.asarray(a, np.float32))

    common = dict(Wl1=bf(W_l1), bl1=f32c(b_l1).reshape(1, -1),
                  Wl2=bf(W_l2), bl2=f32c(b_l2).reshape(1, -1))
    branch_w = {
        0: dict(W1=bf(W_enc1), b1=bf(b_enc1).reshape(1, -1),
                W2=bf(W_r1g1), b2=bf(b_r1g1).reshape(1, -1),
                W3=bf(W_r1g2), b3=bf(b_r1g2).reshape(1, -1)),
        1: dict(W1=bf(W_enc2), b1=bf(b_enc2).reshape(1, -1),
                W2=bf(W_r2g1), b2=bf(b_r2g1).reshape(1, -1),
                W3=bf(W_r2g2), b3=bf(b_r2g2).reshape(1, -1)),
    }
    # x padded to U_TOTAL, bf16
    xpads = []
    for br in (brs, brt):
        xp = np.zeros((U_TOTAL, D_IN), np.float32)
        xp[:N_NODES] = br["xs"]
        xpads.append(np.ascontiguousarray(xp.astype(ml_dtypes.bfloat16)))

    # pack all cores; collect per-core block permutations first
    U_CORE = U_BLOCKS * 128
    packs = []
    for core in range(NCORES):
        bi = 0 if core < CPBRANCH else 1
        br = (brs, brt)[bi]
        c = core % CPBRANCH
        T12, idx12, perm12 = _pack_core(br["eo12"], br["bc12"], br["src"], br["dst"],
                                        br["norm"], c, U_BLOCKS, sched12)
        T3, idx3raw, perm3 = _pack_core(br["eo3"], br["bc3"], br["src"], br["pdst"],
                                        br["norm"], c, p_blocks, sched3)
        packs.append((bi, c, T12, idx12, perm12, T3, idx3raw, perm3))

    # raw row -> permuted storage row maps, per branch (U space and P segs)
    umap = {}
    psegmap = {}
    for bi in range(2):
        um = np.zeros(U_TOTAL, np.int32)
        pm = np.zeros(CPBRANCH * NSEG, np.int32)
        for (bj, c, _, _, perm12, _, _, perm3) in packs:
            if bj != bi:
                continue
            # perm12[pos] = global U block stored at position pos of core c
            for pos, gb in enumerate(perm12):
                rows = np.arange(128)
                um[gb * 128 + rows] = c * U_CORE + pos * 128 + rows
            for pos, gb in enumerate(perm3):
                segs = np.arange(128 // SEG)
                pm[gb * (128 // SEG) + segs] = c * NSEG + pos * (128 // SEG) + segs
        umap[bi] = um
        psegmap[bi] = pm

    # fold indices in permuted seg space (identical on every core)
    fidx = np.full((128, 2 * J), SEGALL, np.int32)
    for bi, br in enumerate((brs, brt)):
        segbase = bi * CPBRANCH * NSEG
        for g in range(G_GRAPHS):
            ns = int(-(-br["counts"][g] // SEG))
            s0 = int(br["pstart"][g]) // SEG
            for j in range(ns):
                fidx[g, bi * J + j] = segbase + psegmap[bi][s0 + j]
    common["fidx"] = fidx

    in_maps = []
    for (bi, c, T12, idx12, perm12, T3, idx3raw, perm3) in packs:
        br = (brs, brt)[bi]
        idx12p = umap[bi][idx12]
        idx3 = umap[bi][idx3raw]
        # valid-row flags, permuted position order for this core
        fl_raw = np.zeros(CPBRANCH * p_core, np.float32)
        for g in range(G_GRAPHS):
            s = int(br["pstart"][g])
            fl_raw[s:s + int(br["counts"][g])] = 1.0
        fl = np.zeros(p_core, np.float32)
        for pos, gb in enumerate(perm3):
            fl[pos * 128:(pos + 1) * 128] = fl_raw[gb * 128:(gb + 1) * 128]
        in_maps.append(dict(
            x=xpads[bi], T12=T12, idx12=idx12, idx12p=idx12p, T3=T3, idx3=idx3,
            fl3=np.ascontiguousarray(fl.astype(ml_dtypes.bfloat16)).reshape(1, -1),
            **branch_w[bi], **common))
    key = (tuple(int(s) for s in sched12), tuple(int(s) for s in sched3),
           int(p_blocks), int(J))
    return in_maps, key


def kernel(**inputs):
    from concourse import bass2jax
    in_maps, key = make_in_maps(**inputs)
    nc = _get_program(key)
    results = bass2jax.run_bass_via_pjrt(nc, in_maps, n_cores=NCORES)
    z = results[0]["out_z"]
    sig = results[0]["out_sig"]
    return (z, sig)
